# revision 23
# baseline (speedup 1.0000x reference)
"""Multi-head attention (B=4, S=2048, D=1024, 16 heads x 64) on 8 trn2 cores.

Sharding: core c handles batch b = c//2 and head-group hg = c%2 (8 heads each,
i.e. columns hg*512:(hg+1)*512 of Wq/Wk/Wv and rows of Wo).  Each core returns
a partial output [S, D]; the host sums the two partials per batch and adds bo.

Per-core kernel, structured so the ScalarE (exp) stream starts as early as
possible and stays saturated -- the 33.6M exp()/core on ACT at 1 elem/cycle
(~219us) is the pacing engine; all PE work that is off the exp critical path
(V/Q projections, out-projection) is emitted as filler the Tile scheduler
slots into exp-wait gaps:

  emission order (stream-aligned; priority = emission order): kb0 slices of
  KT(t0)/QT(c0) -> first score group -> remaining KT chunks, each followed
  by the score/exp groups it unblocks -> V chunks interleaved with z
  accumulation of head-pair 0 and scores of head-pair 1 -> rest of c0 ->
  [QT(c) ; attention(c) ; outproj(c-1)] for c=1..3 -> outproj(3)

  attention(hp, c): per 2-key-tile group: 4 score matmuls emitted
  d0/d1-alternating (lhsT base partitions 0/64 -> distinct PE row groups ->
  the pair runs concurrently on HW), exp fused into PSUM->SBUF eviction on
  ScalarE (scale=1/8, bf16 out), then zT accumulation (K=128) over key
  tiles; V~ carries a ones column (65-wide head groups) so the softmax
  denominator rides along in row 64.  Normalize: reciprocal (DVE) ->
  partition-broadcast (GpSimd) -> multiply into ZT.

  PSUM budget (8 banks): score pair 2x[128,2,512]=4, zT 2x[65,512]=2,
  projections 1, out-projection 1.
"""

import numpy as np

import concourse.bass as bass
import concourse.tile as tile
from concourse import bacc, mybir
from concourse.bass_utils import run_bass_kernel_spmd

F32 = mybir.dt.float32
F32R = mybir.dt.float32r
BF16 = mybir.dt.bfloat16
ACT = mybir.ActivationFunctionType

D = 1024          # d_model
HH = 512          # heads-per-core * head_dim = 8 * 64
HD = 64           # head dim
NHL = 8           # heads per core
B, S_FULL = 4, 2048
N_CORES = 8


def build_nc(S=S_FULL, reps=1, pair=True, v11=True):
    """Build the per-core Bass program (same program for all 8 cores).

    reps > 1 repeats the whole computation inside one program (for timing:
    per-exec HW time is the slope between two reps variants)."""
    nc = bacc.Bacc("TRN2", target_bir_lowering=False, debug=False,
                   dynamic_dma_scratch_size=2048)

    xqT = nc.dram_tensor("xqT", [D, S], BF16, kind="ExternalInput").ap()
    xkT = nc.dram_tensor("xkT", [D, S], BF16, kind="ExternalInput").ap()
    xvT = nc.dram_tensor("xvT", [D, S], BF16, kind="ExternalInput").ap()
    wq = nc.dram_tensor("wq", [D, HH], BF16, kind="ExternalInput").ap()
    wk = nc.dram_tensor("wk", [D, HH], BF16, kind="ExternalInput").ap()
    wv = nc.dram_tensor("wv", [D, HH], BF16, kind="ExternalInput").ap()
    wo = nc.dram_tensor("wo", [HH, D], BF16, kind="ExternalInput").ap()
    bq = nc.dram_tensor("bq", [HH], F32, kind="ExternalInput").ap()
    bk = nc.dram_tensor("bk", [HH], F32, kind="ExternalInput").ap()
    bv = nc.dram_tensor("bv", [HH], F32, kind="ExternalInput").ap()
    out = nc.dram_tensor("out", [S, D], F32, kind="ExternalOutput").ap()

    NT = S // 512        # 512-token chunks
    NSK = S // 128       # 128-token key tiles
    NKT = D // 128       # 128-wide d_model tiles
    NKB = HH // 128      # 128-wide hidden tiles (also head pairs)
    NG = NSK // 2        # 2-key-tile score groups

    with tile.TileContext(nc) as tc:
        from contextlib import ExitStack

        for rep in range(reps):
          with ExitStack() as ctx:
            persist = ctx.enter_context(tc.tile_pool(name="persist", bufs=1))
            wpool = ctx.enter_context(tc.tile_pool(name="wpool", bufs=3))
            xpool = ctx.enter_context(
                tc.tile_pool(name="xpool", bufs=2 if v11 else 3))
            ptpool = ctx.enter_context(
                tc.tile_pool(name="ptpool", bufs=16 if v11 else 12))
            rpool = ctx.enter_context(tc.tile_pool(name="rpool", bufs=2))
            opool = ctx.enter_context(tc.tile_pool(name="opool", bufs=2))

            qt_sb = persist.tile([128, NKB, S], BF16, tag="qt")
            kt_sb = persist.tile([128, NKB, S], BF16, tag="kt")
            vb_sb = persist.tile([128, NSK, NHL * (HD + 1)], BF16, tag="vb")
            zt_sb = persist.tile([128, NKB, S], BF16, tag="zt")
            wo_sb = persist.tile([128, NKB, D], BF16, tag="wo")
            bq_sb = persist.tile([128, NKB], F32, tag="bq")
            bk_sb = persist.tile([128, NKB], F32, tag="bk")
            bvb_sb = persist.tile([128, HH], F32, tag="bvb")

            # bias/V-bias DMAs are emitted later, interleaved with the first
            # projection chunks, so the wk + first-xt DMAs (which gate the
            # first matmul) get the HBM bandwidth first.
            bv_bcast_in = bass.AP(tensor=bv.tensor, offset=bv.offset,
                                  ap=[[0, 128], [1, HH]])
            # ones columns of V~ (softmax denominator trick)
            ones_view = vb_sb.rearrange("p s (h dd) -> p s h dd", dd=HD + 1)[:, :, :, HD:HD + 1]
            nc.vector.memset(ones_view, 1.0)

            def load_w(w_dram, name):
                w_sb = wpool.tile([128, NKT, HH], BF16, tag="w", name=name)
                nc.sync.dma_start(
                    out=w_sb,
                    in_=w_dram.rearrange("(kt p) n -> p kt n", p=128))
                return w_sb

            def proj_chunk(xT, w_sb, dst, bias, t, pool, bufs_tag,
                           kbs=None, xt=None):
                """One 512-token chunk of a K/Q projection into dst[:, :, t].
                kbs selects a subset of the 128-wide output blocks (for
                splitting the critical first chunks); returns the xt tile so
                a follow-up call can reuse it."""
                if xt is None:
                    xt = xpool.tile([128, NKT, 512], BF16, tag="xt", name="xt")
                    nc.sync.dma_start(
                        out=xt,
                        in_=xT.rearrange("(kt p) s -> p kt s", p=128)[:, :, t * 512:(t + 1) * 512])
                for kb in (range(NKB) if kbs is None else kbs):
                    ps = pool.tile([128, 512], F32, tag=bufs_tag, name="ps")
                    for kt in range(NKT):
                        nc.tensor.matmul(
                            ps,
                            lhsT=w_sb[:, kt, kb * 128:(kb + 1) * 128],
                            rhs=xt[:, kt, :],
                            start=(kt == 0), stop=(kt == NKT - 1))
                    nc.vector.tensor_scalar_add(
                        dst[:, kb, t * 512:(t + 1) * 512], ps,
                        bias[:, kb:kb + 1])
                return xt

            def v_chunk(wv_sb, t, pool, bufs_tag):
                """One 512-token chunk of the V projection (natural layout,
                65-wide head groups) into vb_sb."""
                xt = xpool.tile([128, NKT, 512], BF16, tag="xt")
                nc.sync.dma_start(
                    out=xt,
                    in_=xvT.rearrange("(kt p) s -> p kt s", p=128)[:, :, t * 512:(t + 1) * 512])
                for m in range(4):
                    ps = pool.tile([128, 512], F32, tag=bufs_tag)
                    for kt in range(NKT):
                        nc.tensor.matmul(
                            ps,
                            lhsT=xt[:, kt, m * 128:(m + 1) * 128],
                            rhs=wv_sb[:, kt, :],
                            start=(kt == 0), stop=(kt == NKT - 1))
                    sk = t * 4 + m
                    vdst = vb_sb[:, sk, :].rearrange(
                        "p (h dd) -> p h dd", dd=HD + 1)[:, :, 0:HD]
                    nc.vector.tensor_add(
                        vdst,
                        ps.rearrange("p (h d) -> p h d", d=HD),
                        bvb_sb.rearrange("p (h d) -> p h d", d=HD))

            def alloc_zps(hp, c, zpool):
                return [zpool.tile([HD + 1, 512], F32, tag=f"z{d}",
                                   name=f"zps{d}_{hp}_{c}_{rep}", bufs=1)
                        for d in range(2)]

            def scores_exp(hp, c, glo, ghi, spool):
                """Score pairs + exp for groups [glo, ghi); returns the bf16
                probs tiles for z accumulation."""
                out_pts = []
                for g in range(glo, ghi):
                    sps = [spool.tile([128, 2, 512], F32, tag=f"s{d}", bufs=1,
                                      name=f"sp{d}")
                           for d in range(2)]
                    pts = [ptpool.tile([128, 2, 512], BF16, tag=f"pt{d}",
                                       name=f"pt{d}")
                           for d in range(2)]
                    order = ([(2 * g + j, d) for j in range(2) for d in range(2)]
                             if pair else
                             [(2 * g + j, d) for d in range(2) for j in range(2)])
                    for sk, d in order:
                        nc.tensor.matmul(
                            sps[d][:, sk - 2 * g, :],
                            lhsT=kt_sb[d * 64:(d + 1) * 64, hp,
                                       sk * 128:(sk + 1) * 128],
                            rhs=qt_sb[d * 64:(d + 1) * 64, hp,
                                      c * 512:(c + 1) * 512],
                            start=True, stop=True)
                    for d in range(2):
                        nc.scalar.activation(pts[d], sps[d], ACT.Exp,
                                             scale=0.125)
                    out_pts.append(pts)
                return out_pts

            def z_part(hp, glo, ghi, zps, pts_list):
                for g in range(glo, ghi):
                    pts = pts_list[g - glo]
                    for d in range(2):
                        h = 2 * hp + d
                        for j in range(2):
                            sk = 2 * g + j
                            nc.tensor.matmul(
                                zps[d],
                                lhsT=vb_sb[:, sk, h * (HD + 1):(h + 1) * (HD + 1)],
                                rhs=pts[d][:, j, :],
                                start=(sk == 0), stop=(sk == NSK - 1))

            def norm(hp, c, zps):
                for d in range(2):
                    # evict z~ to SBUF so the PSUM bank frees; normalize
                    # from SBUF.  custom-DVE recip can't read base_partition
                    # 64: stage the denominator row at partition 0 first.
                    zr = rpool.tile([HD + 1, 512], F32, tag="zr")
                    nc.vector.tensor_copy(zr, zps[d])
                    dn = rpool.tile([1, 512], F32, tag="dn")
                    nc.vector.tensor_copy(dn, zr[HD:HD + 1, :])
                    rc = rpool.tile([1, 512], F32, tag="rc")
                    nc.vector.reciprocal_approx_fast(rc, dn)
                    bc = rpool.tile([HD, 512], F32, tag="bc")
                    nc.gpsimd.partition_broadcast(bc, rc, channels=HD)
                    nc.vector.tensor_mul(
                        zt_sb[d * 64:d * 64 + HD, hp, c * 512:(c + 1) * 512],
                        zr[0:HD, :], bc)

            def attention(hp, c, spool, zpool):
                """Heads 2hp,2hp+1 x query chunk c: scores (paired), exp, zT."""
                zps = alloc_zps(hp, c, zpool)
                for g in range(NG):
                    pts = scores_exp(hp, c, g, g + 1, spool)
                    z_part(hp, g, g + 1, zps, pts)
                norm(hp, c, zps)

            def outproj(c, pools_tags):
                """Out-projection for query chunk c's 4 token tiles;
                pools_tags is a list of (pool, tag) PSUM slots to rotate
                through (2 slots -> matmul/evict pipelining)."""
                for t4 in range(4):
                    t = c * 4 + t4
                    os_t = opool.tile([128, D], F32, tag="os")
                    for n in range(D // 512):
                        pool, bufs_tag = pools_tags[(2 * t4 + n) % len(pools_tags)]
                        po = pool.tile([128, 512], F32, tag=bufs_tag, bufs=1,
                                       name="po")
                        for hb in range(NKB):
                            nc.tensor.matmul(
                                po,
                                lhsT=zt_sb[:, hb, t * 128:(t + 1) * 128],
                                rhs=wo_sb[:, hb, n * 512:(n + 1) * 512],
                                start=(hb == 0), stop=(hb == NKB - 1))
                        nc.vector.tensor_copy(os_t[:, n * 512:(n + 1) * 512], po)
                    nc.sync.dma_start(out=out[t * 128:(t + 1) * 128, :], in_=os_t)

            # ---- emission plan (priority = emission order):
            # score/z PSUM pools open up-front (6 banks) so attention(c0) can
            # start the moment KT(t0)+QT(c0) land; the 2-bank p1a covers the
            # lead-in projections, then hands its banks to pp/pout.  V is
            # interleaved with the remaining KT chunks so the c0 z matmuls
            # unblock early and recycle the bf16 probs tiles the exp stream
            # needs.  attention(c0) is emitted at priority 0 so the PE
            # prefers score matmuls over projection filler the moment their
            # operands are ready.
            with ExitStack() as c2:
                spool = c2.enter_context(
                    tc.tile_pool(name="spool", bufs=1, space="PSUM"))
                zpool = c2.enter_context(
                    tc.tile_pool(name="zpool", bufs=1, space="PSUM"))

                nc.sync.dma_start(out=bk_sb, in_=bk.rearrange("(kb p) -> p kb", p=128))
                wk_sb = load_w(wk, "w_k")
                with tc.tile_pool(name="p1a", bufs=2, space="PSUM") as p1a:
                    # stream-aligned emission: each KT/V chunk is followed by
                    # the attention work it unblocks, so the PE alternates
                    # producer and score matmuls in priority order and the
                    # exp stream starts ~20us in and never starves.
                    # critical path to the first exp: kb=0 slices of KT(t0)
                    # and QT(c0) only, then score group 0 of head-pair 0.
                    nc.sync.dma_start(out=bq_sb, in_=bq.rearrange("(kb p) -> p kb", p=128))
                    wq_sb = load_w(wq, "w_q")
                    xk0 = proj_chunk(xkT, wk_sb, kt_sb, bk_sb, 0, p1a, "ps1",
                                     kbs=[0])
                    xq0 = proj_chunk(xqT, wq_sb, qt_sb, bq_sb, 0, p1a, "ps1",
                                     kbs=[0])
                    p0 = scores_exp(0, 0, 0, 2, spool)
                    proj_chunk(xkT, wk_sb, kt_sb, bk_sb, 0, p1a, "ps1",
                               kbs=[1, 2, 3], xt=xk0)
                    proj_chunk(xqT, wq_sb, qt_sb, bq_sb, 0, p1a, "ps1",
                               kbs=[1, 2, 3], xt=xq0)
                    nc.sync.dma_start(out=bvb_sb, in_=bv_bcast_in)
                    wv_sb = load_w(wv, "w_v")
                    proj_chunk(xkT, wk_sb, kt_sb, bk_sb, 1, p1a, "ps1")
                    p0 += scores_exp(0, 0, 2, 4, spool)
                    proj_chunk(xkT, wk_sb, kt_sb, bk_sb, 2, p1a, "ps1")
                    p0 += scores_exp(0, 0, 4, 6, spool)
                    proj_chunk(xkT, wk_sb, kt_sb, bk_sb, 3, p1a, "ps1")
                    p0 += scores_exp(0, 0, 6, 8, spool)
                    zps0 = alloc_zps(0, 0, zpool)
                    p1 = []
                    v_chunk(wv_sb, 0, p1a, "ps1")
                    z_part(0, 0, 2, zps0, p0[0:2])
                    p1 += scores_exp(1, 0, 0, 2, spool)
                    v_chunk(wv_sb, 1, p1a, "ps1")
                    z_part(0, 2, 4, zps0, p0[2:4])
                    p1 += scores_exp(1, 0, 2, 4, spool)
                    v_chunk(wv_sb, 2, p1a, "ps1")
                    z_part(0, 4, 6, zps0, p0[4:6])
                    p1 += scores_exp(1, 0, 4, 6, spool)
                    v_chunk(wv_sb, 3, p1a, "ps1")
                    z_part(0, 6, 8, zps0, p0[6:8])
                    p1 += scores_exp(1, 0, 6, 8, spool)
                    if v11:
                        # QT(c1) inside the production window: chunk-1
                        # score/exp groups become ACT fodder while the PE
                        # produces V/KT and chunk-0's z matmuls wait on V.
                        proj_chunk(xqT, wq_sb, qt_sb, bq_sb, 1, p1a, "ps1")

                pp = c2.enter_context(
                    tc.tile_pool(name="pp", bufs=1, space="PSUM"))
                pout = c2.enter_context(
                    tc.tile_pool(name="pout", bufs=1, space="PSUM"))
                nc.sync.dma_start(
                    out=wo_sb,
                    in_=wo.rearrange("(hb p) n -> p hb n", p=128))

                norm(0, 0, zps0)
                zps1 = alloc_zps(1, 0, zpool)
                z_part(1, 0, 8, zps1, p1)
                norm(1, 0, zps1)
                attention(2, 0, spool, zpool)
                attention(3, 0, spool, zpool)

                for c in range(1, NT):
                    tproj = c + 1 if v11 else c
                    if tproj < NT:
                        proj_chunk(xqT, wq_sb, qt_sb, bq_sb, tproj, pp, "p1b")
                    for hp in range(NKB):
                        attention(hp, c, spool, zpool)
                    outproj(c - 1, [(pout, "po")])
                # z banks are free after the last chunk's normalize: rotate
                # the final out-projection over 4 banks for deeper pipelining
                outproj(NT - 1, [(pout, "po"), (pp, "p1b"),
                                 (zpool, "z0"), (zpool, "z1")])

    nc.compile()
    return nc


_NC_CACHE = {}


def _get_nc(S=S_FULL):
    if S not in _NC_CACHE:
        _NC_CACHE[S] = build_nc(S)
    return _NC_CACHE[S]


def make_in_maps(query, key, value, Wq, bq, Wk, bk, Wv, bv, Wo, bo):
    """Shard full inputs into 8 per-core input dicts."""
    f32 = lambda a: np.ascontiguousarray(np.asarray(a, dtype=np.float32))
    bf16_dt = mybir.dt.np(BF16)
    bf16 = lambda a: np.ascontiguousarray(
        np.asarray(a, dtype=np.float32).astype(bf16_dt))
    in_maps = []
    for core in range(N_CORES):
        b, hg = core // 2, core % 2
        sl = slice(hg * HH, (hg + 1) * HH)
        in_maps.append({
            "xqT": bf16(np.asarray(query)[b].T),
            "xkT": bf16(np.asarray(key)[b].T),
            "xvT": bf16(np.asarray(value)[b].T),
            "wq": bf16(np.asarray(Wq)[:, sl]),
            "wk": bf16(np.asarray(Wk)[:, sl]),
            "wv": bf16(np.asarray(Wv)[:, sl]),
            "wo": bf16(np.asarray(Wo)[sl, :]),
            "bq": f32(np.asarray(bq)[sl]),
            "bk": f32(np.asarray(bk)[sl]),
            "bv": f32(np.asarray(bv)[sl]),
        })
    return in_maps


def kernel(query, key, value, Wq, bq, Wk, bk, Wv, bv, Wo, bo, **run_kwargs):
    nc = _get_nc(S_FULL)
    in_maps = make_in_maps(query, key, value, Wq, bq, Wk, bk, Wv, bv, Wo, bo)
    res = run_bass_kernel_spmd(nc, in_maps, core_ids=list(range(N_CORES)),
                               **run_kwargs)
    bo_np = np.asarray(bo, dtype=np.float32)
    outs = [np.asarray(r["out"], dtype=np.float32) for r in res.results]
    full = np.stack([outs[2 * b] + outs[2 * b + 1] + bo_np for b in range(B)])
    return full.astype(np.float32)
